# revision 17
# baseline (speedup 1.0000x reference)
import numpy as np
import ml_dtypes

import concourse.tile as tile
from concourse import bacc, mybir
from concourse.bass_utils import run_bass_kernel_spmd

L, D_IN, D_HID, D_OUT, NTOT = 8, 256, 1024, 256, 32768
W = 512                       # max tokens per block (SBUF/PSUM tile width)
BLKS = [512] * 7 + [256, 256]  # small final blocks shorten the tail chain
COFF = [sum(BLKS[:i]) for i in range(len(BLKS) + 1)]
NBLK = len(BLKS)
P = COFF[-1]                  # 4096 padded tokens per core (one plane per core)
KI = D_IN // 128              # 2
MJ = D_HID // 128             # 8
MO = D_OUT // 128             # 2

F32 = mybir.dt.float32
BF16 = mybir.dt.bfloat16
FP8 = mybir.dt.float8e4
AF = mybir.ActivationFunctionType
DRMODE = mybir.MatmulPerfMode.DoubleRow
NPBF16 = ml_dtypes.bfloat16
NPFP8 = ml_dtypes.float8_e4m3

# First NF8 hidden slices (js) run L1 as fp8 DoubleRow matmuls (2x PE rate).
# Error budget: measured rel_err 1.73e-2 with NF8=2 vs the 2e-2 gate.
NF8 = 2
S1 = 2048.0                   # W1 pre-scale so fp8 values sit in e4m3 normal range
INV_S1 = float(1.0 / S1)

WARMUP_MMS = 5                # dummy matmuls during initial DMA wait to warm the PE clock

PROFILE = False
LAST_RES = None
_nc_cache = None


def _build_nc():
    nc = bacc.Bacc()
    xp_d = nc.declare_dram_parameter("xp", [128, KI * P], BF16, isOutput=False)
    w1p_d = nc.declare_dram_parameter("w1p", [128, MJ * KI * 128], BF16, isOutput=False)
    w2p_d = nc.declare_dram_parameter("w2p", [128, MO * MJ * 128], BF16, isOutput=False)
    b1p_d = nc.declare_dram_parameter("b1p", [128, MJ], F32, isOutput=False)
    b2p_d = nc.declare_dram_parameter("b2p", [128, MO], F32, isOutput=False)
    outp_d = nc.declare_dram_parameter("outp", [128, MO * P], BF16, isOutput=True)
    xq_d = nc.declare_dram_parameter("xq", [128, KI, P], FP8, isOutput=False)
    w1q_d = nc.declare_dram_parameter("w1q", [128, KI, NF8 * 128], FP8, isOutput=False)

    with tile.TileContext(nc) as tc:
        with (
            tc.tile_pool(name="wpool", bufs=1) as wp,
            tc.tile_pool(name="xr", bufs=4) as xrp,
            tc.tile_pool(name="hr", bufs=2) as hrp,
            tc.tile_pool(name="outp", bufs=2) as outp,
            tc.tile_pool(name="ps1", bufs=6, space="PSUM") as ps1,
            tc.tile_pool(name="ps2", bufs=1, space="PSUM") as ps2,
        ):
            def x_load(ib, k1_eng=None):
                # streaming blocks keep every issue on the sync queue so the
                # scalar queue stays free for gelu dispatch; startup blocks
                # pass k1_eng=nc.scalar for issue parallelism
                w = BLKS[ib]
                xq = xrp.tile([128, KI, W], FP8, tag="xq")
                nc.sync.dma_start(xq[:, :, 0:w], xq_d[:, :, COFF[ib]:COFF[ib] + w])
                ts = []
                for k in range(KI):
                    r = xrp.tile([128, W], BF16, tag=f"x{k}")
                    o = KI * COFF[ib] + k * w
                    eng = nc.sync if k == 0 else (k1_eng or nc.sync)
                    eng.dma_start(r[:, 0:w], xp_d[:, o:o + w])
                    ts.append(r)
                return ts, xq

            # ---- critical-path DMA emission ----
            # sync ring:   w1j0, x0k0, w1j3, w1m(j4-5), w2, x-k0 stream,
            #              out-i1 stream, tail out-i0
            # scalar ring: x0k1, w1a(j1-2), b1, w1b(j6-7) (4 quick pushes, then
            #              the ACT table load + gelus own the sequencer),
            #              x-k1 stream, tail out-i1
            # gpsimd:      mid-stream out-i0 only (fire-and-forget SWDGE)
            w1q_t = wp.tile([128, KI, NF8 * 128], FP8, tag="w1q")   # j0,j1 fp8
            nc.sync.dma_start(w1q_t[:], w1q_d[:])
            xb0, xq0 = x_load(0, k1_eng=nc.scalar)
            xb1, xq1 = x_load(1, k1_eng=nc.scalar)
            w1a = wp.tile([128, 2 * KI * 128], BF16, tag="w1a")     # j2,j3
            nc.scalar.dma_start(w1a[:], w1p_d[:, 2 * KI * 128:4 * KI * 128])
            w1m = wp.tile([128, 2 * KI * 128], BF16, tag="w1m")     # j4,j5
            nc.sync.dma_start(w1m[:], w1p_d[:, 4 * KI * 128:6 * KI * 128])
            b1_t = wp.tile([128, MJ], F32, tag="b1")
            nc.scalar.dma_start(b1_t[:], b1p_d[:])
            w1b = wp.tile([128, 2 * KI * 128], BF16, tag="w1b")     # j6,j7
            nc.scalar.dma_start(w1b[:], w1p_d[:, 6 * KI * 128:MJ * KI * 128])

            def w1_sl(j, k):
                if j <= 3:
                    o = ((j - 2) * KI + k) * 128
                    return w1a[:, o:o + 128]
                if j <= 5:
                    o = ((j - 4) * KI + k) * 128
                    return w1m[:, o:o + 128]
                o = ((j - 6) * KI + k) * 128
                return w1b[:, o:o + 128]

            def l1_mms(pt, j, w, xr, xqr):
                if j < NF8:
                    nc.tensor.matmul(pt[:, 0:w], w1q_t[:, :, j * 128:(j + 1) * 128],
                                     xqr[:, :, 0:w], start=True, stop=True,
                                     perf_mode=DRMODE)
                else:
                    for k in range(KI):
                        nc.tensor.matmul(pt[:, 0:w], w1_sl(j, k), xr[k][:, 0:w],
                                         start=(k == 0), stop=(k == KI - 1))
            # w2 halves: tiles allocated here, DMA issued in-loop (at ib=0)
            # so the 512KB doesn't queue ahead of x0/x1/w1 at startup.
            w2h = [[wp.tile([128, 4 * 128], BF16, tag=f"w2i{i}h{h}",
                            name=f"w2i{i}h{h}") for h in range(2)]
                   for i in range(MO)]

            def w2_sl(i, j):
                return w2h[i][j // 4][:, (j % 4) * 128:(j % 4 + 1) * 128]

            # ---- PE warm-up: matmuls with no DMA dependency fill the init window ----
            if WARMUP_MMS:
                wdum = wp.tile([128, 128], BF16, tag="wdum")
                xdum = wp.tile([128, W], BF16, tag="xdum")
                nc.vector.memset(wdum[:], 0.0)
                nc.vector.memset(xdum[:], 0.0)
                # preload the Gelu activation table during the DMA wait window.
                # Must mirror the real call's operand form (bias as AP) or
                # walrus re-emits the table load before the first real gelu.
                actd = wp.tile([128, 1], F32, tag="actd")
                nc.vector.memset(actd[:], 0.0)
                nc.scalar.activation(actd[:], actd[:], AF.Gelu, bias=actd[:, 0:1])
                psd = ps1.tile([128, W], F32, tag="h")
                for m in range(WARMUP_MMS):
                    nc.tensor.matmul(psd[:], wdum[:], xdum[:],
                                     start=(m == 0), stop=(m == WARMUP_MMS - 1))

            # Fused L1 for blocks 0+1: each w1 piece feeds both blocks (4 MMs
            # per j instead of 2), halving the weight-arrival rate the PE needs
            # during the receipt-bound startup window.
            hr01 = [{}, {}]
            xb01 = [xb0, xb1]
            xq01 = [xq0, xq1]
            for j in range(MJ):
                for b in range(2):
                    wb = BLKS[b]
                    pt = ps1.tile([128, W], F32, tag="h", name="pt01")
                    l1_mms(pt, j, wb, xb01[b], xq01[b])
                    h = hrp.tile([128, W], BF16, tag=f"h{j}", name=f"h{j}b{b}")
                    nc.scalar.activation(h[:, 0:wb], pt[:, 0:wb], AF.Gelu,
                                         bias=b1_t[:, j:j + 1],
                                         scale=(INV_S1 if j < NF8 else 1.0))
                    hr01[b][j] = h

            xcur = (xb0, xq0)
            xnxt = (xb1, xq1)
            for ib in range(NBLK):
                w = BLKS[ib]
                xr, xqr = xcur
                xcur = xnxt
                xnxt = x_load(ib + 2) if ib + 2 < NBLK else None
                if ib == 0:
                    # issue w2 now: j0-3 halves first (needed first by L2)
                    for hf in range(2):
                        for i in range(MO):
                            o = (i * MJ + hf * 4) * 128
                            nc.sync.dma_start(w2h[i][hf][:],
                                              w2p_d[:, o:o + 4 * 128])
                if ib < 2:
                    hr = hr01[ib]
                else:
                    hr = []
                    for j in range(MJ):
                        pt = ps1.tile([128, W], F32, tag="h")
                        l1_mms(pt, j, w, xr, xqr)
                        h = hrp.tile([128, W], BF16, tag=f"h{j}")
                        nc.scalar.activation(h[:, 0:w], pt[:, 0:w], AF.Gelu,
                                             bias=b1_t[:, j:j + 1],
                                             scale=(INV_S1 if j < NF8 else 1.0))
                        hr.append(h)
                ot = outp.tile([128, MO * W], BF16, tag="o")
                # j-outer / i-inner: both output banks accumulate in parallel so
                # each gelu h[j] is consumed at 2 MMs per step (more slack for ACT)
                pt2s = [ps2.tile([128, W], F32, tag=f"o{i}", name=f"o{i}")
                        for i in range(MO)]
                for j in range(MJ):
                    for i in range(MO):
                        nc.tensor.matmul(pt2s[i][:, 0:w], w2_sl(i, j),
                                         hr[j][:, 0:w], start=(j == 0), stop=(j == MJ - 1))
                # b2 is added host-side; drains only move PSUM -> SBUF (bf16).
                # Final block: drains split across vector and scalar engines,
                # pushes on both warm HWDGE rings, to shorten the tail chain.
                last = ib == NBLK - 1
                for i in range(MO):
                    od = outp_d[:, MO * COFF[ib] + i * w:MO * COFF[ib] + (i + 1) * w]
                    if last and i == 1:
                        nc.scalar.activation(ot[:, i * w:(i + 1) * w],
                                             pt2s[i][:, 0:w], AF.Copy)
                        nc.scalar.dma_start(od, ot[:, i * w:(i + 1) * w])
                    else:
                        nc.vector.tensor_scalar_add(ot[:, i * w:(i + 1) * w],
                                                    pt2s[i][:, 0:w], 0.0)
                        if last:
                            eng = nc.sync
                        else:
                            eng = nc.gpsimd if i == 0 else nc.sync
                        eng.dma_start(od, ot[:, i * w:(i + 1) * w])
    if not nc.is_finalized():
        nc.finalize()
    return nc


def _erf(z):
    # Abramowitz & Stegun 7.1.26, |err| <= 1.5e-7
    s = np.sign(z)
    z = np.abs(z)
    t = 1.0 / (1.0 + 0.3275911 * z)
    y = 1.0 - (((((1.061405429 * t - 1.453152027) * t) + 1.421413741) * t
                - 0.284496736) * t + 0.254829592) * t * np.exp(-z * z)
    return s * y


def _mlp_f64(xo, W1c, b1c, W2c, b2c):
    h = xo.astype(np.float64) @ W1c.T.astype(np.float64) + b1c.astype(np.float64)
    g = 0.5 * h * (1.0 + _erf(h / np.sqrt(2.0)))
    return (g @ W2c.T.astype(np.float64) + b2c.astype(np.float64)).astype(np.float32)


def kernel(x, W1, b1, W2, b2, plane_idx):
    global _nc_cache, LAST_RES
    x = np.ascontiguousarray(x, dtype=np.float32)
    W1 = np.asarray(W1, dtype=np.float32)
    b1 = np.asarray(b1, dtype=np.float32)
    W2 = np.asarray(W2, dtype=np.float32)
    b2 = np.asarray(b2, dtype=np.float32)
    plane_idx = np.asarray(plane_idx)

    order = np.argsort(plane_idx, kind="stable")
    counts = np.bincount(plane_idx, minlength=L)
    starts = np.concatenate([[0], np.cumsum(counts)])

    in_maps = []
    idxs = []
    for c in range(L):
        idx = order[starts[c]:starts[c + 1]]
        idxs.append(idx)
        n = min(len(idx), P)
        xt32 = np.zeros((D_IN, P), dtype=np.float32)
        xt32[:, :n] = x[idx[:n]].T
        xtb = xt32.astype(NPBF16)
        xq3 = np.ascontiguousarray(
            xt32.reshape(KI, 128, P).transpose(1, 0, 2).astype(NPFP8))
        xp = np.empty((128, KI * P), dtype=NPBF16)
        for ib in range(NBLK):
            w = BLKS[ib]
            for k in range(KI):
                o = KI * COFF[ib] + k * w
                xp[:, o:o + w] = xtb[k * 128:(k + 1) * 128, COFF[ib]:COFF[ib] + w]
        w1p = np.ascontiguousarray(
            W1[c].T.reshape(KI, 128, MJ, 128).transpose(1, 2, 0, 3)
            .reshape(128, MJ * KI * 128).astype(NPBF16))
        w1q = np.ascontiguousarray(
            (W1[c][:NF8 * 128, :].T * np.float32(S1))
            .reshape(KI, 128, NF8 * 128).transpose(1, 0, 2).astype(NPFP8))
        w2p = np.ascontiguousarray(
            W2[c].T.reshape(MJ, 128, MO, 128).transpose(1, 2, 0, 3)
            .reshape(128, MO * MJ * 128).astype(NPBF16))
        in_maps.append({
            "xp": xp,
            "w1p": w1p,
            "w2p": w2p,
            "b1p": np.ascontiguousarray(b1[c].reshape(MJ, 128).T),
            "b2p": np.ascontiguousarray(b2[c].reshape(MO, 128).T),
            "xq": xq3,
            "w1q": w1q,
        })

    if _nc_cache is None:
        _nc_cache = _build_nc()
    res = run_bass_kernel_spmd(_nc_cache, in_maps, list(range(L)), trace=PROFILE)
    LAST_RES = res

    out = np.empty((x.shape[0], D_OUT), dtype=np.float32)
    for c in range(L):
        idx = idxs[c]
        n = min(len(idx), P)
        op = np.asarray(res.results[c]["outp"]).astype(np.float32)
        outT = np.empty((D_OUT, P), dtype=np.float32)
        for ib in range(NBLK):
            w = BLKS[ib]
            for i in range(MO):
                o = MO * COFF[ib] + i * w
                outT[i * 128:(i + 1) * 128, COFF[ib]:COFF[ib] + w] = op[:, o:o + w]
        out[idx[:n]] = outT[:, :n].T + b2[c][None, :]
        if len(idx) > n:
            out[idx[n:]] = _mlp_f64(x[idx[n:]], W1[c], b1[c], W2[c], b2[c])
    return out



# revision 27
# speedup vs baseline: 1.0034x; 1.0034x over previous
import numpy as np
import ml_dtypes

import concourse.tile as tile
from concourse import bacc, mybir
from concourse.bass_utils import run_bass_kernel_spmd

L, D_IN, D_HID, D_OUT, NTOT = 8, 256, 1024, 256, 32768
W = 512                       # max tokens per block (SBUF/PSUM tile width)
BLKS = [512] * 7 + [256, 256]  # small final blocks shorten the tail chain
COFF = [sum(BLKS[:i]) for i in range(len(BLKS) + 1)]
NBLK = len(BLKS)
P = COFF[-1]                  # 4096 padded tokens per core (one plane per core)
KI = D_IN // 128              # 2
MJ = D_HID // 128             # 8
MO = D_OUT // 128             # 2

F32 = mybir.dt.float32
BF16 = mybir.dt.bfloat16
FP8 = mybir.dt.float8e4
AF = mybir.ActivationFunctionType
DRMODE = mybir.MatmulPerfMode.DoubleRow
NPBF16 = ml_dtypes.bfloat16
NPFP8 = ml_dtypes.float8_e4m3

# First NF8 hidden slices (js) run L1 as fp8 DoubleRow matmuls (2x PE rate).
# Error budget: measured rel_err 1.73e-2 with NF8=2 vs the 2e-2 gate.
NF8 = 2
S1 = 2048.0                   # W1 pre-scale so fp8 values sit in e4m3 normal range
INV_S1 = float(1.0 / S1)

WARMUP_MMS = 5                # dummy matmuls during initial DMA wait to warm the PE clock

PROFILE = False
LAST_RES = None
_nc_cache = None


def _build_nc():
    nc = bacc.Bacc()
    xp_d = nc.declare_dram_parameter("xp", [128, KI * P], BF16, isOutput=False)
    w1p_d = nc.declare_dram_parameter("w1p", [128, MJ * KI * 128], BF16, isOutput=False)
    w2p_d = nc.declare_dram_parameter("w2p", [128, MO * MJ * 128], BF16, isOutput=False)
    b1p_d = nc.declare_dram_parameter("b1p", [128, MJ], F32, isOutput=False)
    b2p_d = nc.declare_dram_parameter("b2p", [128, MO], F32, isOutput=False)
    outp_d = nc.declare_dram_parameter("outp", [128, MO * P], BF16, isOutput=True)
    xq_d = nc.declare_dram_parameter("xq", [128, KI * P], FP8, isOutput=False)
    w1q_d = nc.declare_dram_parameter("w1q", [128, KI, NF8 * 128], FP8, isOutput=False)

    with tile.TileContext(nc) as tc:
        with (
            tc.tile_pool(name="wpool", bufs=1) as wp,
            tc.tile_pool(name="xr", bufs=4) as xrp,
            tc.tile_pool(name="hr", bufs=2) as hrp,
            tc.tile_pool(name="outp", bufs=2) as outp,
            tc.tile_pool(name="ps1", bufs=6, space="PSUM") as ps1,
            tc.tile_pool(name="ps2", bufs=1, space="PSUM") as ps2,
        ):
            def x_load(ib, k1_eng=None):
                # streaming blocks keep every issue on the sync queue so the
                # scalar queue stays free for gelu dispatch; startup blocks
                # pass k1_eng=nc.scalar for issue parallelism
                w = BLKS[ib]
                o0 = KI * COFF[ib]
                xq = xrp.tile([128, KI * W], FP8, tag="xq")
                nc.sync.dma_start(xq[:, 0:KI * w], xq_d[:, o0:o0 + KI * w])
                ts = []
                for k in range(KI):
                    r = xrp.tile([128, W], BF16, tag=f"x{k}")
                    o = o0 + k * w
                    eng = nc.sync if k == 0 else (k1_eng or nc.sync)
                    eng.dma_start(r[:, 0:w], xp_d[:, o:o + w])
                    ts.append(r)
                return ts, xq

            # ---- critical-path DMA emission ----
            # sync ring:   w1j0, x0k0, w1j3, w1m(j4-5), w2, x-k0 stream,
            #              out-i1 stream, tail out-i0
            # scalar ring: x0k1, w1a(j1-2), b1, w1b(j6-7) (4 quick pushes, then
            #              the ACT table load + gelus own the sequencer),
            #              x-k1 stream, tail out-i1
            # gpsimd:      mid-stream out-i0 only (fire-and-forget SWDGE)
            # Startup DMA priority: the rings round-robin descriptors, so
            # every queued transfer completes together at total-bytes/BW.
            # Queue as little as possible ahead of the first-needed data;
            # later weights (w1b, w2) are issued from inside the loop.
            w1q_t = wp.tile([128, KI, NF8 * 128], FP8, tag="w1q")   # j0,j1 fp8
            nc.sync.dma_start(w1q_t[:], w1q_d[:])
            xb0, xq0 = x_load(0, k1_eng=nc.scalar)
            b1_t = wp.tile([128, MJ], F32, tag="b1")
            nc.scalar.dma_start(b1_t[:], b1p_d[:])
            xb1, xq1 = x_load(1, k1_eng=nc.scalar)
            w1a = wp.tile([128, 2 * KI * 128], BF16, tag="w1a")     # j2,j3
            nc.scalar.dma_start(w1a[:], w1p_d[:, 2 * KI * 128:4 * KI * 128])
            w1m = wp.tile([128, 2 * KI * 128], BF16, tag="w1m")     # j4,j5
            nc.sync.dma_start(w1m[:], w1p_d[:, 4 * KI * 128:6 * KI * 128])
            w1b = wp.tile([128, 2 * KI * 128], BF16, tag="w1b")     # j6,j7
            nc.scalar.dma_start(w1b[:], w1p_d[:, 6 * KI * 128:MJ * KI * 128])

            def w1_sl(j, k):
                if j <= 3:
                    o = ((j - 2) * KI + k) * 128
                    return w1a[:, o:o + 128]
                if j <= 5:
                    o = ((j - 4) * KI + k) * 128
                    return w1m[:, o:o + 128]
                o = ((j - 6) * KI + k) * 128
                return w1b[:, o:o + 128]

            def l1_mms(pt, j, w, xr, xqr):
                if j < NF8:
                    rhs = xqr[:, 0:KI * w].rearrange("p (k n) -> p k n", k=KI)
                    nc.tensor.matmul(pt[:, 0:w], w1q_t[:, :, j * 128:(j + 1) * 128],
                                     rhs, start=True, stop=True,
                                     perf_mode=DRMODE)
                else:
                    for k in range(KI):
                        nc.tensor.matmul(pt[:, 0:w], w1_sl(j, k), xr[k][:, 0:w],
                                         start=(k == 0), stop=(k == KI - 1))
            # w2 halves: tiles allocated here, DMA issued in-loop (at ib=0)
            # so the 512KB doesn't queue ahead of x0/x1/w1 at startup.
            w2h = [[wp.tile([128, 4 * 128], BF16, tag=f"w2i{i}h{h}",
                            name=f"w2i{i}h{h}") for h in range(2)]
                   for i in range(MO)]

            def w2_sl(i, j):
                return w2h[i][j // 4][:, (j % 4) * 128:(j % 4 + 1) * 128]

            # ---- PE warm-up: matmuls with no DMA dependency fill the init window ----
            if WARMUP_MMS:
                wdum = wp.tile([128, 128], BF16, tag="wdum")
                xdum = wp.tile([128, W], BF16, tag="xdum")
                nc.vector.memset(wdum[:], 0.0)
                nc.vector.memset(xdum[:], 0.0)
                # preload the Gelu activation table during the DMA wait window.
                # Must mirror the real call's operand form (bias as AP) or
                # walrus re-emits the table load before the first real gelu.
                actd = wp.tile([128, 1], F32, tag="actd")
                nc.vector.memset(actd[:], 0.0)
                nc.scalar.activation(actd[:], actd[:], AF.Gelu, bias=actd[:, 0:1])
                psd = ps1.tile([128, W], F32, tag="h")
                for m in range(WARMUP_MMS):
                    nc.tensor.matmul(psd[:], wdum[:], xdum[:],
                                     start=(m == 0), stop=(m == WARMUP_MMS - 1))

            # Fused L1 for blocks 0+1: each w1 piece feeds both blocks (4 MMs
            # per j instead of 2), halving the weight-arrival rate the PE needs
            # during the receipt-bound startup window.
            hr01 = [{}, {}]
            xb01 = [xb0, xb1]
            xq01 = [xq0, xq1]
            for j in range(MJ):
                for b in range(2):
                    wb = BLKS[b]
                    pt = ps1.tile([128, W], F32, tag="h", name="pt01")
                    l1_mms(pt, j, wb, xb01[b], xq01[b])
                    h = hrp.tile([128, W], BF16, tag=f"h{j}", name=f"h{j}b{b}")
                    nc.scalar.activation(h[:, 0:wb], pt[:, 0:wb], AF.Gelu,
                                         bias=b1_t[:, j:j + 1],
                                         scale=(INV_S1 if j < NF8 else 1.0))
                    hr01[b][j] = h

            xcur = (xb0, xq0)
            xnxt = (xb1, xq1)
            for ib in range(NBLK):
                w = BLKS[ib]
                xr, xqr = xcur
                xcur = xnxt
                if ib == 0:
                    # late weights: w2 (j0-3 halves first), ahead of the x2
                    # prefetch on the sync queue
                    for hf in range(2):
                        for i in range(MO):
                            o = (i * MJ + hf * 4) * 128
                            nc.sync.dma_start(w2h[i][hf][:],
                                              w2p_d[:, o:o + 4 * 128])
                xnxt = x_load(ib + 2) if ib + 2 < NBLK else None
                if ib < 2:
                    hr = hr01[ib]
                else:
                    hr = []
                    for j in range(MJ):
                        pt = ps1.tile([128, W], F32, tag="h")
                        l1_mms(pt, j, w, xr, xqr)
                        h = hrp.tile([128, W], BF16, tag=f"h{j}")
                        nc.scalar.activation(h[:, 0:w], pt[:, 0:w], AF.Gelu,
                                             bias=b1_t[:, j:j + 1],
                                             scale=(INV_S1 if j < NF8 else 1.0))
                        hr.append(h)
                ot = outp.tile([128, MO * W], BF16, tag="o")
                # j-outer / i-inner: both output banks accumulate in parallel so
                # each gelu h[j] is consumed at 2 MMs per step (more slack for ACT)
                pt2s = [ps2.tile([128, W], F32, tag=f"o{i}", name=f"o{i}")
                        for i in range(MO)]
                for j in range(MJ):
                    for i in range(MO):
                        nc.tensor.matmul(pt2s[i][:, 0:w], w2_sl(i, j),
                                         hr[j][:, 0:w], start=(j == 0), stop=(j == MJ - 1))
                # b2 is added host-side; drains only move PSUM -> SBUF (bf16).
                # Final block: drains split across vector and scalar engines,
                # pushes on both warm HWDGE rings, to shorten the tail chain.
                last = ib == NBLK - 1
                for i in range(MO):
                    od = outp_d[:, MO * COFF[ib] + i * w:MO * COFF[ib] + (i + 1) * w]
                    nc.vector.tensor_scalar_add(ot[:, i * w:(i + 1) * w],
                                                pt2s[i][:, 0:w], 0.0)
                    if last:
                        eng = nc.sync if i == 0 else nc.scalar
                    else:
                        eng = nc.gpsimd if i == 0 else nc.sync
                    eng.dma_start(od, ot[:, i * w:(i + 1) * w])
    if not nc.is_finalized():
        nc.finalize()
    return nc


def _erf(z):
    # Abramowitz & Stegun 7.1.26, |err| <= 1.5e-7
    s = np.sign(z)
    z = np.abs(z)
    t = 1.0 / (1.0 + 0.3275911 * z)
    y = 1.0 - (((((1.061405429 * t - 1.453152027) * t) + 1.421413741) * t
                - 0.284496736) * t + 0.254829592) * t * np.exp(-z * z)
    return s * y


def _mlp_f64(xo, W1c, b1c, W2c, b2c):
    h = xo.astype(np.float64) @ W1c.T.astype(np.float64) + b1c.astype(np.float64)
    g = 0.5 * h * (1.0 + _erf(h / np.sqrt(2.0)))
    return (g @ W2c.T.astype(np.float64) + b2c.astype(np.float64)).astype(np.float32)


def kernel(x, W1, b1, W2, b2, plane_idx):
    global _nc_cache, LAST_RES
    x = np.ascontiguousarray(x, dtype=np.float32)
    W1 = np.asarray(W1, dtype=np.float32)
    b1 = np.asarray(b1, dtype=np.float32)
    W2 = np.asarray(W2, dtype=np.float32)
    b2 = np.asarray(b2, dtype=np.float32)
    plane_idx = np.asarray(plane_idx)

    order = np.argsort(plane_idx, kind="stable")
    counts = np.bincount(plane_idx, minlength=L)
    starts = np.concatenate([[0], np.cumsum(counts)])

    in_maps = []
    idxs = []
    for c in range(L):
        idx = order[starts[c]:starts[c + 1]]
        idxs.append(idx)
        n = min(len(idx), P)
        xt32 = np.zeros((D_IN, P), dtype=np.float32)
        xt32[:, :n] = x[idx[:n]].T
        xtb = xt32.astype(NPBF16)
        xt8 = xt32.astype(NPFP8)
        xp = np.empty((128, KI * P), dtype=NPBF16)
        xq8 = np.empty((128, KI * P), dtype=NPFP8)
        for ib in range(NBLK):
            w = BLKS[ib]
            for k in range(KI):
                o = KI * COFF[ib] + k * w
                xp[:, o:o + w] = xtb[k * 128:(k + 1) * 128, COFF[ib]:COFF[ib] + w]
                xq8[:, o:o + w] = xt8[k * 128:(k + 1) * 128, COFF[ib]:COFF[ib] + w]
        w1p = np.ascontiguousarray(
            W1[c].T.reshape(KI, 128, MJ, 128).transpose(1, 2, 0, 3)
            .reshape(128, MJ * KI * 128).astype(NPBF16))
        w1q = np.ascontiguousarray(
            (W1[c][:NF8 * 128, :].T * np.float32(S1))
            .reshape(KI, 128, NF8 * 128).transpose(1, 0, 2).astype(NPFP8))
        w2p = np.ascontiguousarray(
            W2[c].T.reshape(MJ, 128, MO, 128).transpose(1, 2, 0, 3)
            .reshape(128, MO * MJ * 128).astype(NPBF16))
        in_maps.append({
            "xp": xp,
            "w1p": w1p,
            "w2p": w2p,
            "b1p": np.ascontiguousarray(b1[c].reshape(MJ, 128).T),
            "b2p": np.ascontiguousarray(b2[c].reshape(MO, 128).T),
            "xq": xq8,
            "w1q": w1q,
        })

    if _nc_cache is None:
        _nc_cache = _build_nc()
    res = run_bass_kernel_spmd(_nc_cache, in_maps, list(range(L)), trace=PROFILE)
    LAST_RES = res

    out = np.empty((x.shape[0], D_OUT), dtype=np.float32)
    for c in range(L):
        idx = idxs[c]
        n = min(len(idx), P)
        op = np.asarray(res.results[c]["outp"]).astype(np.float32)
        outT = np.empty((D_OUT, P), dtype=np.float32)
        for ib in range(NBLK):
            w = BLKS[ib]
            for i in range(MO):
                o = MO * COFF[ib] + i * w
                outT[i * 128:(i + 1) * 128, COFF[ib]:COFF[ib] + w] = op[:, o:o + w]
        out[idx[:n]] = outT[:, :n].T + b2[c][None, :]
        if len(idx) > n:
            out[idx[n:]] = _mlp_f64(x[idx[n:]], W1[c], b1[c], W2[c], b2[c])
    return out



# revision 29
# speedup vs baseline: 1.0139x; 1.0105x over previous
import numpy as np
import ml_dtypes

import concourse.tile as tile
from concourse import bacc, mybir
from concourse.bass_utils import run_bass_kernel_spmd

L, D_IN, D_HID, D_OUT, NTOT = 8, 256, 1024, 256, 32768
W = 512                       # max tokens per block (SBUF/PSUM tile width)
BLKS = [512] * 7 + [256, 256]  # small final blocks shorten the tail chain
COFF = [sum(BLKS[:i]) for i in range(len(BLKS) + 1)]
NBLK = len(BLKS)
P = COFF[-1]                  # 4096 padded tokens per core (one plane per core)
KI = D_IN // 128              # 2
MJ = D_HID // 128             # 8
MO = D_OUT // 128             # 2

F32 = mybir.dt.float32
BF16 = mybir.dt.bfloat16
FP8 = mybir.dt.float8e4
AF = mybir.ActivationFunctionType
DRMODE = mybir.MatmulPerfMode.DoubleRow
NPBF16 = ml_dtypes.bfloat16
NPFP8 = ml_dtypes.float8_e4m3

# First NF8 hidden slices (js) run L1 as fp8 DoubleRow matmuls (2x PE rate).
# Error budget: measured rel_err 1.73e-2 with NF8=2 vs the 2e-2 gate.
NF8 = 2
S1 = 2048.0                   # W1 pre-scale so fp8 values sit in e4m3 normal range
INV_S1 = float(1.0 / S1)

WARMUP_MMS = 4                # dummy matmuls during initial DMA wait to warm the PE clock

PROFILE = False
LAST_RES = None
_nc_cache = None


def _build_nc():
    nc = bacc.Bacc()
    xp_d = nc.declare_dram_parameter("xp", [128, KI * P], BF16, isOutput=False)
    w1p_d = nc.declare_dram_parameter("w1p", [128, MJ * KI * 128], BF16, isOutput=False)
    w2p_d = nc.declare_dram_parameter("w2p", [128, MO * MJ * 128], BF16, isOutput=False)
    b1p_d = nc.declare_dram_parameter("b1p", [128, MJ], F32, isOutput=False)
    b2p_d = nc.declare_dram_parameter("b2p", [128, MO], F32, isOutput=False)
    outp_d = nc.declare_dram_parameter("outp", [128, MO * P], BF16, isOutput=True)
    xq_d = nc.declare_dram_parameter("xq", [128, KI * P], FP8, isOutput=False)
    w1q_d = nc.declare_dram_parameter("w1q", [128, KI, NF8 * 128], FP8, isOutput=False)

    with tile.TileContext(nc) as tc:
        with (
            tc.tile_pool(name="wpool", bufs=1) as wp,
            tc.tile_pool(name="xr", bufs=4) as xrp,
            tc.tile_pool(name="hr", bufs=2) as hrp,
            tc.tile_pool(name="outp", bufs=2) as outp,
            tc.tile_pool(name="ps1", bufs=6, space="PSUM") as ps1,
            tc.tile_pool(name="ps2", bufs=1, space="PSUM") as ps2,
        ):
            def x_load(ib, k1_eng=None):
                # streaming blocks keep every issue on the sync queue so the
                # scalar queue stays free for gelu dispatch; startup blocks
                # pass k1_eng=nc.scalar for issue parallelism
                w = BLKS[ib]
                o0 = KI * COFF[ib]
                xq = xrp.tile([128, KI * W], FP8, tag="xq")
                nc.sync.dma_start(xq[:, 0:KI * w], xq_d[:, o0:o0 + KI * w])
                ts = []
                for k in range(KI):
                    r = xrp.tile([128, W], BF16, tag=f"x{k}")
                    o = o0 + k * w
                    eng = nc.sync if k == 0 else (k1_eng or nc.sync)
                    eng.dma_start(r[:, 0:w], xp_d[:, o:o + w])
                    ts.append(r)
                return ts, xq

            # ---- critical-path DMA emission ----
            # sync ring:   w1j0, x0k0, w1j3, w1m(j4-5), w2, x-k0 stream,
            #              out-i1 stream, tail out-i0
            # scalar ring: x0k1, w1a(j1-2), b1, w1b(j6-7) (4 quick pushes, then
            #              the ACT table load + gelus own the sequencer),
            #              x-k1 stream, tail out-i1
            # gpsimd:      mid-stream out-i0 only (fire-and-forget SWDGE)
            # Startup DMA priority: the rings round-robin descriptors, so
            # every queued transfer completes together at total-bytes/BW.
            # Queue as little as possible ahead of the first-needed data;
            # later weights (w1b, w2) are issued from inside the loop.
            w1q_t = wp.tile([128, KI, NF8 * 128], FP8, tag="w1q")   # j0,j1 fp8
            nc.sync.dma_start(w1q_t[:], w1q_d[:])
            xb0, xq0 = x_load(0, k1_eng=nc.scalar)
            b1_t = wp.tile([128, MJ], F32, tag="b1")
            nc.scalar.dma_start(b1_t[:], b1p_d[:])
            xb1, xq1 = x_load(1, k1_eng=nc.scalar)
            w1a = wp.tile([128, 2 * KI * 128], BF16, tag="w1a")     # j2,j3
            nc.scalar.dma_start(w1a[:], w1p_d[:, 2 * KI * 128:4 * KI * 128])
            w1m = wp.tile([128, 2 * KI * 128], BF16, tag="w1m")     # j4,j5
            nc.sync.dma_start(w1m[:], w1p_d[:, 4 * KI * 128:6 * KI * 128])
            w1b = wp.tile([128, 2 * KI * 128], BF16, tag="w1b")     # j6,j7
            nc.scalar.dma_start(w1b[:], w1p_d[:, 6 * KI * 128:MJ * KI * 128])

            def w1_sl(j, k):
                if j <= 3:
                    o = ((j - 2) * KI + k) * 128
                    return w1a[:, o:o + 128]
                if j <= 5:
                    o = ((j - 4) * KI + k) * 128
                    return w1m[:, o:o + 128]
                o = ((j - 6) * KI + k) * 128
                return w1b[:, o:o + 128]

            def l1_mms(pt, j, w, xr, xqr):
                if j < NF8:
                    rhs = xqr[:, 0:KI * w].rearrange("p (k n) -> p k n", k=KI)
                    nc.tensor.matmul(pt[:, 0:w], w1q_t[:, :, j * 128:(j + 1) * 128],
                                     rhs, start=True, stop=True,
                                     perf_mode=DRMODE)
                else:
                    for k in range(KI):
                        nc.tensor.matmul(pt[:, 0:w], w1_sl(j, k), xr[k][:, 0:w],
                                         start=(k == 0), stop=(k == KI - 1))
            # w2 halves: tiles allocated here, DMA issued in-loop (at ib=0)
            # so the 512KB doesn't queue ahead of x0/x1/w1 at startup.
            w2h = [[wp.tile([128, 4 * 128], BF16, tag=f"w2i{i}h{h}",
                            name=f"w2i{i}h{h}") for h in range(2)]
                   for i in range(MO)]

            def w2_sl(i, j):
                return w2h[i][j // 4][:, (j % 4) * 128:(j % 4 + 1) * 128]

            # ---- PE warm-up: matmuls with no DMA dependency fill the init window ----
            if WARMUP_MMS:
                wdum = wp.tile([128, 128], BF16, tag="wdum")
                xdum = wp.tile([128, W], BF16, tag="xdum")
                nc.vector.memset(wdum[:], 0.0)
                nc.vector.memset(xdum[:], 0.0)
                # preload the Gelu activation table during the DMA wait window.
                # Must mirror the real call's operand form (bias as AP) or
                # walrus re-emits the table load before the first real gelu.
                actd = wp.tile([128, 1], F32, tag="actd")
                nc.vector.memset(actd[:], 0.0)
                nc.scalar.activation(actd[:], actd[:], AF.Gelu, bias=actd[:, 0:1])
                psd = ps1.tile([128, W], F32, tag="h")
                for m in range(WARMUP_MMS):
                    nc.tensor.matmul(psd[:], wdum[:], xdum[:],
                                     start=(m == 0), stop=(m == WARMUP_MMS - 1))

            # Fused L1 for blocks 0+1: each w1 piece feeds both blocks (4 MMs
            # per j instead of 2), halving the weight-arrival rate the PE needs
            # during the receipt-bound startup window.
            # Fused L1 for blocks 0+1, block-outer: the in-order tensor queue
            # must not hit block 1's x dependencies until block 0 is emitted,
            # else one late DMA stalls the PE and resets the HAM clock window.
            hr01 = [{}, {}]
            xb01 = [xb0, xb1]
            xq01 = [xq0, xq1]
            for b in range(2):
                for j in range(MJ):
                    wb = BLKS[b]
                    pt = ps1.tile([128, W], F32, tag="h", name="pt01")
                    l1_mms(pt, j, wb, xb01[b], xq01[b])
                    h = hrp.tile([128, W], BF16, tag=f"h{j}", name=f"h{j}b{b}")
                    nc.scalar.activation(h[:, 0:wb], pt[:, 0:wb], AF.Gelu,
                                         bias=b1_t[:, j:j + 1],
                                         scale=(INV_S1 if j < NF8 else 1.0))
                    hr01[b][j] = h

            xcur = (xb0, xq0)
            xnxt = (xb1, xq1)
            for ib in range(NBLK):
                w = BLKS[ib]
                xr, xqr = xcur
                xcur = xnxt
                if ib == 0:
                    # late weights: w2 (j0-3 halves first), ahead of the x2
                    # prefetch on the sync queue
                    for hf in range(2):
                        for i in range(MO):
                            o = (i * MJ + hf * 4) * 128
                            nc.sync.dma_start(w2h[i][hf][:],
                                              w2p_d[:, o:o + 4 * 128])
                xnxt = x_load(ib + 2) if ib + 2 < NBLK else None
                if ib < 2:
                    hr = hr01[ib]
                else:
                    hr = []
                    for j in range(MJ):
                        pt = ps1.tile([128, W], F32, tag="h")
                        l1_mms(pt, j, w, xr, xqr)
                        h = hrp.tile([128, W], BF16, tag=f"h{j}")
                        nc.scalar.activation(h[:, 0:w], pt[:, 0:w], AF.Gelu,
                                             bias=b1_t[:, j:j + 1],
                                             scale=(INV_S1 if j < NF8 else 1.0))
                        hr.append(h)
                ot = outp.tile([128, MO * W], BF16, tag="o")
                # j-outer / i-inner: both output banks accumulate in parallel so
                # each gelu h[j] is consumed at 2 MMs per step (more slack for ACT)
                pt2s = [ps2.tile([128, W], F32, tag=f"o{i}", name=f"o{i}")
                        for i in range(MO)]
                for j in range(MJ):
                    for i in range(MO):
                        nc.tensor.matmul(pt2s[i][:, 0:w], w2_sl(i, j),
                                         hr[j][:, 0:w], start=(j == 0), stop=(j == MJ - 1))
                # b2 is added host-side; drains only move PSUM -> SBUF (bf16).
                # Final block: drains split across vector and scalar engines,
                # pushes on both warm HWDGE rings, to shorten the tail chain.
                last = ib == NBLK - 1
                for i in range(MO):
                    od = outp_d[:, MO * COFF[ib] + i * w:MO * COFF[ib] + (i + 1) * w]
                    nc.vector.tensor_scalar_add(ot[:, i * w:(i + 1) * w],
                                                pt2s[i][:, 0:w], 0.0)
                    if last:
                        eng = nc.sync if i == 0 else nc.scalar
                    else:
                        eng = nc.gpsimd if i == 0 else nc.sync
                    eng.dma_start(od, ot[:, i * w:(i + 1) * w])
    if not nc.is_finalized():
        nc.finalize()
    return nc


def _erf(z):
    # Abramowitz & Stegun 7.1.26, |err| <= 1.5e-7
    s = np.sign(z)
    z = np.abs(z)
    t = 1.0 / (1.0 + 0.3275911 * z)
    y = 1.0 - (((((1.061405429 * t - 1.453152027) * t) + 1.421413741) * t
                - 0.284496736) * t + 0.254829592) * t * np.exp(-z * z)
    return s * y


def _mlp_f64(xo, W1c, b1c, W2c, b2c):
    h = xo.astype(np.float64) @ W1c.T.astype(np.float64) + b1c.astype(np.float64)
    g = 0.5 * h * (1.0 + _erf(h / np.sqrt(2.0)))
    return (g @ W2c.T.astype(np.float64) + b2c.astype(np.float64)).astype(np.float32)


def kernel(x, W1, b1, W2, b2, plane_idx):
    global _nc_cache, LAST_RES
    x = np.ascontiguousarray(x, dtype=np.float32)
    W1 = np.asarray(W1, dtype=np.float32)
    b1 = np.asarray(b1, dtype=np.float32)
    W2 = np.asarray(W2, dtype=np.float32)
    b2 = np.asarray(b2, dtype=np.float32)
    plane_idx = np.asarray(plane_idx)

    order = np.argsort(plane_idx, kind="stable")
    counts = np.bincount(plane_idx, minlength=L)
    starts = np.concatenate([[0], np.cumsum(counts)])

    in_maps = []
    idxs = []
    for c in range(L):
        idx = order[starts[c]:starts[c + 1]]
        idxs.append(idx)
        n = min(len(idx), P)
        xt32 = np.zeros((D_IN, P), dtype=np.float32)
        xt32[:, :n] = x[idx[:n]].T
        xtb = xt32.astype(NPBF16)
        xt8 = xt32.astype(NPFP8)
        xp = np.empty((128, KI * P), dtype=NPBF16)
        xq8 = np.empty((128, KI * P), dtype=NPFP8)
        for ib in range(NBLK):
            w = BLKS[ib]
            for k in range(KI):
                o = KI * COFF[ib] + k * w
                xp[:, o:o + w] = xtb[k * 128:(k + 1) * 128, COFF[ib]:COFF[ib] + w]
                xq8[:, o:o + w] = xt8[k * 128:(k + 1) * 128, COFF[ib]:COFF[ib] + w]
        w1p = np.ascontiguousarray(
            W1[c].T.reshape(KI, 128, MJ, 128).transpose(1, 2, 0, 3)
            .reshape(128, MJ * KI * 128).astype(NPBF16))
        w1q = np.ascontiguousarray(
            (W1[c][:NF8 * 128, :].T * np.float32(S1))
            .reshape(KI, 128, NF8 * 128).transpose(1, 0, 2).astype(NPFP8))
        w2p = np.ascontiguousarray(
            W2[c].T.reshape(MJ, 128, MO, 128).transpose(1, 2, 0, 3)
            .reshape(128, MO * MJ * 128).astype(NPBF16))
        in_maps.append({
            "xp": xp,
            "w1p": w1p,
            "w2p": w2p,
            "b1p": np.ascontiguousarray(b1[c].reshape(MJ, 128).T),
            "b2p": np.ascontiguousarray(b2[c].reshape(MO, 128).T),
            "xq": xq8,
            "w1q": w1q,
        })

    if _nc_cache is None:
        _nc_cache = _build_nc()
    res = run_bass_kernel_spmd(_nc_cache, in_maps, list(range(L)), trace=PROFILE)
    LAST_RES = res

    out = np.empty((x.shape[0], D_OUT), dtype=np.float32)
    for c in range(L):
        idx = idxs[c]
        n = min(len(idx), P)
        op = np.asarray(res.results[c]["outp"]).astype(np.float32)
        outT = np.empty((D_OUT, P), dtype=np.float32)
        for ib in range(NBLK):
            w = BLKS[ib]
            for i in range(MO):
                o = MO * COFF[ib] + i * w
                outT[i * 128:(i + 1) * 128, COFF[ib]:COFF[ib] + w] = op[:, o:o + w]
        out[idx[:n]] = outT[:, :n].T + b2[c][None, :]
        if len(idx) > n:
            out[idx[n:]] = _mlp_f64(x[idx[n:]], W1[c], b1[c], W2[c], b2[c])
    return out



# revision 30
# speedup vs baseline: 1.0247x; 1.0106x over previous
import numpy as np
import ml_dtypes

import concourse.tile as tile
from concourse import bacc, mybir
from concourse.bass_utils import run_bass_kernel_spmd

L, D_IN, D_HID, D_OUT, NTOT = 8, 256, 1024, 256, 32768
W = 512                       # max tokens per block (SBUF/PSUM tile width)
BLKS = [512] * 7 + [256, 256]  # small final blocks shorten the tail chain
COFF = [sum(BLKS[:i]) for i in range(len(BLKS) + 1)]
NBLK = len(BLKS)
P = COFF[-1]                  # 4096 padded tokens per core (one plane per core)
KI = D_IN // 128              # 2
MJ = D_HID // 128             # 8
MO = D_OUT // 128             # 2

F32 = mybir.dt.float32
BF16 = mybir.dt.bfloat16
FP8 = mybir.dt.float8e4
AF = mybir.ActivationFunctionType
DRMODE = mybir.MatmulPerfMode.DoubleRow
NPBF16 = ml_dtypes.bfloat16
NPFP8 = ml_dtypes.float8_e4m3

# First NF8 hidden slices (js) run L1 as fp8 DoubleRow matmuls (2x PE rate).
# Error budget: measured rel_err 1.73e-2 with NF8=2 vs the 2e-2 gate.
NF8 = 2
S1 = 2048.0                   # W1 pre-scale so fp8 values sit in e4m3 normal range
INV_S1 = float(1.0 / S1)

WARMUP_MMS = 4                # dummy matmuls during initial DMA wait to warm the PE clock

PROFILE = False
LAST_RES = None
_nc_cache = None


def _build_nc():
    nc = bacc.Bacc()
    xp_d = nc.declare_dram_parameter("xp", [128, KI * P], BF16, isOutput=False)
    w1p_d = nc.declare_dram_parameter("w1p", [128, MJ * KI * 128], BF16, isOutput=False)
    w2p_d = nc.declare_dram_parameter("w2p", [128, MO * MJ * 128], BF16, isOutput=False)
    b1p_d = nc.declare_dram_parameter("b1p", [128, MJ], F32, isOutput=False)
    b2p_d = nc.declare_dram_parameter("b2p", [128, MO], F32, isOutput=False)
    outp_d = nc.declare_dram_parameter("outp", [128, MO * P], BF16, isOutput=True)
    xq_d = nc.declare_dram_parameter("xq", [128, KI * P], FP8, isOutput=False)
    w1q_d = nc.declare_dram_parameter("w1q", [128, KI, NF8 * 128], FP8, isOutput=False)

    with tile.TileContext(nc) as tc:
        with (
            tc.tile_pool(name="wpool", bufs=1) as wp,
            tc.tile_pool(name="xr", bufs=4) as xrp,
            tc.tile_pool(name="hr", bufs=2) as hrp,
            tc.tile_pool(name="outp", bufs=2) as outp,
            tc.tile_pool(name="ps1", bufs=6, space="PSUM") as ps1,
            tc.tile_pool(name="ps2", bufs=1, space="PSUM") as ps2,
        ):
            def x_load(ib, k1_eng=None):
                # streaming blocks keep every issue on the sync queue so the
                # scalar queue stays free for gelu dispatch; startup blocks
                # pass k1_eng=nc.scalar for issue parallelism
                w = BLKS[ib]
                o0 = KI * COFF[ib]
                xq = xrp.tile([128, KI * W], FP8, tag="xq")
                nc.sync.dma_start(xq[:, 0:KI * w], xq_d[:, o0:o0 + KI * w])
                ts = []
                for k in range(KI):
                    r = xrp.tile([128, W], BF16, tag=f"x{k}")
                    o = o0 + k * w
                    eng = nc.sync if k == 0 else (k1_eng or nc.sync)
                    eng.dma_start(r[:, 0:w], xp_d[:, o:o + w])
                    ts.append(r)
                return ts, xq

            # ---- critical-path DMA emission ----
            # sync ring:   w1j0, x0k0, w1j3, w1m(j4-5), w2, x-k0 stream,
            #              out-i1 stream, tail out-i0
            # scalar ring: x0k1, w1a(j1-2), b1, w1b(j6-7) (4 quick pushes, then
            #              the ACT table load + gelus own the sequencer),
            #              x-k1 stream, tail out-i1
            # gpsimd:      mid-stream out-i0 only (fire-and-forget SWDGE)
            # Startup DMA priority: the rings round-robin descriptors, so
            # every queued transfer completes together at total-bytes/BW.
            # Queue as little as possible ahead of the first-needed data;
            # later weights (w1b, w2) are issued from inside the loop.
            w1q_t = wp.tile([128, KI, NF8 * 128], FP8, tag="w1q")   # j0,j1 fp8
            nc.sync.dma_start(w1q_t[:], w1q_d[:])
            xb0, xq0 = x_load(0, k1_eng=nc.scalar)
            b1_t = wp.tile([128, MJ], F32, tag="b1")
            nc.scalar.dma_start(b1_t[:], b1p_d[:])
            w1a = wp.tile([128, 2 * KI * 128], BF16, tag="w1a")     # j2,j3
            nc.scalar.dma_start(w1a[:], w1p_d[:, 2 * KI * 128:4 * KI * 128])
            # Gate the second DMA wave behind block-0's arrival: the rings
            # round-robin all queued transfers, so later bytes in flight
            # delay the first-needed ones. The tiny SBUF->SBUF copies block
            # each issue queue until block 0's x has landed.
            gate_t = wp.tile([128, 4], BF16, tag="gate")
            nc.sync.dma_start(gate_t[0:1, 0:2], xb0[0][0:1, 0:2])
            nc.scalar.dma_start(gate_t[0:1, 2:4], xb0[1][0:1, 0:2])
            xb1, xq1 = x_load(1, k1_eng=nc.scalar)
            w1m = wp.tile([128, 2 * KI * 128], BF16, tag="w1m")     # j4,j5
            nc.sync.dma_start(w1m[:], w1p_d[:, 4 * KI * 128:6 * KI * 128])
            w1b = wp.tile([128, 2 * KI * 128], BF16, tag="w1b")     # j6,j7
            nc.scalar.dma_start(w1b[:], w1p_d[:, 6 * KI * 128:MJ * KI * 128])

            def w1_sl(j, k):
                if j <= 3:
                    o = ((j - 2) * KI + k) * 128
                    return w1a[:, o:o + 128]
                if j <= 5:
                    o = ((j - 4) * KI + k) * 128
                    return w1m[:, o:o + 128]
                o = ((j - 6) * KI + k) * 128
                return w1b[:, o:o + 128]

            def l1_mms(pt, j, w, xr, xqr):
                if j < NF8:
                    rhs = xqr[:, 0:KI * w].rearrange("p (k n) -> p k n", k=KI)
                    nc.tensor.matmul(pt[:, 0:w], w1q_t[:, :, j * 128:(j + 1) * 128],
                                     rhs, start=True, stop=True,
                                     perf_mode=DRMODE)
                else:
                    for k in range(KI):
                        nc.tensor.matmul(pt[:, 0:w], w1_sl(j, k), xr[k][:, 0:w],
                                         start=(k == 0), stop=(k == KI - 1))
            # w2 halves: tiles allocated here, DMA issued in-loop (at ib=0)
            # so the 512KB doesn't queue ahead of x0/x1/w1 at startup.
            w2h = [[wp.tile([128, 4 * 128], BF16, tag=f"w2i{i}h{h}",
                            name=f"w2i{i}h{h}") for h in range(2)]
                   for i in range(MO)]

            def w2_sl(i, j):
                return w2h[i][j // 4][:, (j % 4) * 128:(j % 4 + 1) * 128]

            # ---- PE warm-up: matmuls with no DMA dependency fill the init window ----
            if WARMUP_MMS:
                wdum = wp.tile([128, 128], BF16, tag="wdum")
                xdum = wp.tile([128, W], BF16, tag="xdum")
                nc.vector.memset(wdum[:], 0.0)
                nc.vector.memset(xdum[:], 0.0)
                # preload the Gelu activation table during the DMA wait window.
                # Must mirror the real call's operand form (bias as AP) or
                # walrus re-emits the table load before the first real gelu.
                actd = wp.tile([128, 1], F32, tag="actd")
                nc.vector.memset(actd[:], 0.0)
                nc.scalar.activation(actd[:], actd[:], AF.Gelu, bias=actd[:, 0:1])
                psd = ps1.tile([128, W], F32, tag="h")
                for m in range(WARMUP_MMS):
                    nc.tensor.matmul(psd[:], wdum[:], xdum[:],
                                     start=(m == 0), stop=(m == WARMUP_MMS - 1))

            # Fused L1 for blocks 0+1: each w1 piece feeds both blocks (4 MMs
            # per j instead of 2), halving the weight-arrival rate the PE needs
            # during the receipt-bound startup window.
            # Fused L1 for blocks 0+1, block-outer: the in-order tensor queue
            # must not hit block 1's x dependencies until block 0 is emitted,
            # else one late DMA stalls the PE and resets the HAM clock window.
            hr01 = [{}, {}]
            xb01 = [xb0, xb1]
            xq01 = [xq0, xq1]
            for b in range(2):
                for j in range(MJ):
                    wb = BLKS[b]
                    pt = ps1.tile([128, W], F32, tag="h", name="pt01")
                    l1_mms(pt, j, wb, xb01[b], xq01[b])
                    h = hrp.tile([128, W], BF16, tag=f"h{j}", name=f"h{j}b{b}")
                    nc.scalar.activation(h[:, 0:wb], pt[:, 0:wb], AF.Gelu,
                                         bias=b1_t[:, j:j + 1],
                                         scale=(INV_S1 if j < NF8 else 1.0))
                    hr01[b][j] = h

            xcur = (xb0, xq0)
            xnxt = (xb1, xq1)
            for ib in range(NBLK):
                w = BLKS[ib]
                xr, xqr = xcur
                xcur = xnxt
                if ib == 0:
                    # late weights: w2 (j0-3 halves first), ahead of the x2
                    # prefetch on the sync queue
                    for hf in range(2):
                        for i in range(MO):
                            o = (i * MJ + hf * 4) * 128
                            nc.sync.dma_start(w2h[i][hf][:],
                                              w2p_d[:, o:o + 4 * 128])
                xnxt = x_load(ib + 2) if ib + 2 < NBLK else None
                if ib < 2:
                    hr = hr01[ib]
                else:
                    hr = []
                    for j in range(MJ):
                        pt = ps1.tile([128, W], F32, tag="h")
                        l1_mms(pt, j, w, xr, xqr)
                        h = hrp.tile([128, W], BF16, tag=f"h{j}")
                        nc.scalar.activation(h[:, 0:w], pt[:, 0:w], AF.Gelu,
                                             bias=b1_t[:, j:j + 1],
                                             scale=(INV_S1 if j < NF8 else 1.0))
                        hr.append(h)
                ot = outp.tile([128, MO * W], BF16, tag="o")
                # j-outer / i-inner: both output banks accumulate in parallel so
                # each gelu h[j] is consumed at 2 MMs per step (more slack for ACT)
                pt2s = [ps2.tile([128, W], F32, tag=f"o{i}", name=f"o{i}")
                        for i in range(MO)]
                for j in range(MJ):
                    for i in range(MO):
                        nc.tensor.matmul(pt2s[i][:, 0:w], w2_sl(i, j),
                                         hr[j][:, 0:w], start=(j == 0), stop=(j == MJ - 1))
                # b2 is added host-side; drains only move PSUM -> SBUF (bf16).
                # Final block: drains split across vector and scalar engines,
                # pushes on both warm HWDGE rings, to shorten the tail chain.
                last = ib == NBLK - 1
                for i in range(MO):
                    od = outp_d[:, MO * COFF[ib] + i * w:MO * COFF[ib] + (i + 1) * w]
                    nc.vector.tensor_scalar_add(ot[:, i * w:(i + 1) * w],
                                                pt2s[i][:, 0:w], 0.0)
                    if last:
                        eng = nc.sync if i == 0 else nc.scalar
                    else:
                        eng = nc.gpsimd if i == 0 else nc.sync
                    eng.dma_start(od, ot[:, i * w:(i + 1) * w])
    if not nc.is_finalized():
        nc.finalize()
    return nc


def _erf(z):
    # Abramowitz & Stegun 7.1.26, |err| <= 1.5e-7
    s = np.sign(z)
    z = np.abs(z)
    t = 1.0 / (1.0 + 0.3275911 * z)
    y = 1.0 - (((((1.061405429 * t - 1.453152027) * t) + 1.421413741) * t
                - 0.284496736) * t + 0.254829592) * t * np.exp(-z * z)
    return s * y


def _mlp_f64(xo, W1c, b1c, W2c, b2c):
    h = xo.astype(np.float64) @ W1c.T.astype(np.float64) + b1c.astype(np.float64)
    g = 0.5 * h * (1.0 + _erf(h / np.sqrt(2.0)))
    return (g @ W2c.T.astype(np.float64) + b2c.astype(np.float64)).astype(np.float32)


def kernel(x, W1, b1, W2, b2, plane_idx):
    global _nc_cache, LAST_RES
    x = np.ascontiguousarray(x, dtype=np.float32)
    W1 = np.asarray(W1, dtype=np.float32)
    b1 = np.asarray(b1, dtype=np.float32)
    W2 = np.asarray(W2, dtype=np.float32)
    b2 = np.asarray(b2, dtype=np.float32)
    plane_idx = np.asarray(plane_idx)

    order = np.argsort(plane_idx, kind="stable")
    counts = np.bincount(plane_idx, minlength=L)
    starts = np.concatenate([[0], np.cumsum(counts)])

    in_maps = []
    idxs = []
    for c in range(L):
        idx = order[starts[c]:starts[c + 1]]
        idxs.append(idx)
        n = min(len(idx), P)
        xt32 = np.zeros((D_IN, P), dtype=np.float32)
        xt32[:, :n] = x[idx[:n]].T
        xtb = xt32.astype(NPBF16)
        xt8 = xt32.astype(NPFP8)
        xp = np.empty((128, KI * P), dtype=NPBF16)
        xq8 = np.empty((128, KI * P), dtype=NPFP8)
        for ib in range(NBLK):
            w = BLKS[ib]
            for k in range(KI):
                o = KI * COFF[ib] + k * w
                xp[:, o:o + w] = xtb[k * 128:(k + 1) * 128, COFF[ib]:COFF[ib] + w]
                xq8[:, o:o + w] = xt8[k * 128:(k + 1) * 128, COFF[ib]:COFF[ib] + w]
        w1p = np.ascontiguousarray(
            W1[c].T.reshape(KI, 128, MJ, 128).transpose(1, 2, 0, 3)
            .reshape(128, MJ * KI * 128).astype(NPBF16))
        w1q = np.ascontiguousarray(
            (W1[c][:NF8 * 128, :].T * np.float32(S1))
            .reshape(KI, 128, NF8 * 128).transpose(1, 0, 2).astype(NPFP8))
        w2p = np.ascontiguousarray(
            W2[c].T.reshape(MJ, 128, MO, 128).transpose(1, 2, 0, 3)
            .reshape(128, MO * MJ * 128).astype(NPBF16))
        in_maps.append({
            "xp": xp,
            "w1p": w1p,
            "w2p": w2p,
            "b1p": np.ascontiguousarray(b1[c].reshape(MJ, 128).T),
            "b2p": np.ascontiguousarray(b2[c].reshape(MO, 128).T),
            "xq": xq8,
            "w1q": w1q,
        })

    if _nc_cache is None:
        _nc_cache = _build_nc()
    res = run_bass_kernel_spmd(_nc_cache, in_maps, list(range(L)), trace=PROFILE)
    LAST_RES = res

    out = np.empty((x.shape[0], D_OUT), dtype=np.float32)
    for c in range(L):
        idx = idxs[c]
        n = min(len(idx), P)
        op = np.asarray(res.results[c]["outp"]).astype(np.float32)
        outT = np.empty((D_OUT, P), dtype=np.float32)
        for ib in range(NBLK):
            w = BLKS[ib]
            for i in range(MO):
                o = MO * COFF[ib] + i * w
                outT[i * 128:(i + 1) * 128, COFF[ib]:COFF[ib] + w] = op[:, o:o + w]
        out[idx[:n]] = outT[:, :n].T + b2[c][None, :]
        if len(idx) > n:
            out[idx[n:]] = _mlp_f64(x[idx[n:]], W1[c], b1[c], W2[c], b2[c])
    return out

